# revision 26
# baseline (speedup 1.0000x reference)
"""Trainium2 Bass kernel for multi-head attention (BS=2048, D=1024, H=16, d_k=64).

Returns (output [2048,1024], attn [16,2048,2048]) like the reference.

Sharding: tensor-parallel over heads -- each of the 8 cores owns 2 heads.
Each core reads the full (host-pretransposed) q/k/v plus its head-slices of
the weights, computes its 2 heads' attention + attn output, writes its slice
of `attn` and a partial output projection.  Host sums the 8 partials and adds
the bias constants (bo + Wo@bv, which factor out exactly).

Per-core dataflow (all matmuls in float32r = full PE speed):
  - qhT/khT [128(head dims),2048] = W @ x^T projections (PSUM accum over 8
    k-chunks of D_IN, bias added on ScalarE eviction).
  - vh in natural [k-row, d] layout (lhsT = vT chunks), with a ones column
    appended so attn@V also produces softmax row sums.
  - phase N (per head, 16 q-strips): S = qhT^T @ khT -> PSUM [128,2048];
    ScalarE exp(0.125*S) with fused accum_out row-sums; VectorE reciprocal +
    tensor_scalar (per-partition) normalize; DMA the finished attn strip out.
  - phase T (per head, 2 q-halves, 16 k-strips): S^T = khT^T @ qhT; exp;
    attn@V accumulates O' [65,1024] over k-strips (row 64 = row sums);
    broadcast 1/s via a K=1 ones matmul, normalize O' on eviction.
  - output projection: out_part = O^T(both heads) @ WoT, accumulated as two
    K=64 matmuls per tile, evicted + DMA'd.
"""

import os
import sys
from contextlib import ExitStack

if "/opt/trn_rl_repo" not in sys.path:
    sys.path.insert(0, "/opt/trn_rl_repo")

import numpy as np

BS = 2048
D_IN = 1024
D_OUT = 1024
H = 16
D_K = 64
N_CORES = 8
H_LOC = H // N_CORES          # 2 heads per core
HD = H_LOC * D_K              # 128 head dims per core
KCH = D_IN // 128             # 8 contraction chunks for projections
NSTRIP = BS // 128            # 16 strips of 128
SCALE = 1.0 / np.sqrt(D_K)    # 0.125

_CACHE = {}

# Filled by the last run (for test.py): bass_utils.BassKernelResults
LAST_RESULTS = None


def _build_bass():
    import concourse.bass as bass
    import concourse.tile as tile
    import concourse.mybir as mybir
    from concourse import bacc

    f32 = mybir.dt.float32
    f16 = mybir.dt.float16
    AF = mybir.ActivationFunctionType

    nc = bacc.Bacc(None, target_bir_lowering=False)

    qT = nc.dram_tensor("qT", [4, 128, KCH, 512], f16, kind="ExternalInput")
    kT = nc.dram_tensor("kT", [4, 128, KCH, 512], f16, kind="ExternalInput")
    vT = nc.dram_tensor("vT", [4, 128, KCH, 512], f16, kind="ExternalInput")
    wqT = nc.dram_tensor("wqT", [D_IN, HD], f16, kind="ExternalInput")
    wkT = nc.dram_tensor("wkT", [D_IN, HD], f16, kind="ExternalInput")
    wvT = nc.dram_tensor("wvT", [D_IN, HD], f16, kind="ExternalInput")
    woT = nc.dram_tensor("woT", [HD, D_OUT], f16, kind="ExternalInput")
    onesd = nc.dram_tensor("ones", [128, 128], f16, kind="ExternalInput")
    bq = nc.dram_tensor("bq", [HD, 1], f32, kind="ExternalInput")
    bk = nc.dram_tensor("bk", [HD, 1], f32, kind="ExternalInput")

    attn_out = nc.dram_tensor("attn", [H_LOC, BS, BS], f32, kind="ExternalOutput")
    outp = nc.dram_tensor("outp", [BS, D_OUT], f32, kind="ExternalOutput")

    QB = 512                     # q-quarter width for the T stream
    NQB = BS // QB               # 4 quarters

    with tile.TileContext(nc) as tc, ExitStack() as ctx:
        consts = ctx.enter_context(tc.tile_pool(name="consts", bufs=1))
        slabs = ctx.enter_context(tc.tile_pool(name="slabs", bufs=3))
        vspool = ctx.enter_context(tc.tile_pool(name="vslabs", bufs=4))
        upool = ctx.enter_context(tc.tile_pool(name="u", bufs=6))
        utpool = ctx.enter_context(tc.tile_pool(name="ut", bufs=10))
        apool = ctx.enter_context(tc.tile_pool(name="a", bufs=4))
        spool = ctx.enter_context(tc.tile_pool(name="s", bufs=8))
        outpool = ctx.enter_context(tc.tile_pool(name="outsb", bufs=3))
        orawpool = ctx.enter_context(tc.tile_pool(name="oraw", bufs=3))
        rrowpool = ctx.enter_context(tc.tile_pool(name="rrow", bufs=6))

        # 8-bank PSUM budget:
        #   pn: 2 x [128,1024]f32 (4 banks) -- N scores (one slot per head),
        #       also vh-psum [128,8,128] and outproj [128,1024]
        #   pt: 2 x [128,512]f32 (2 banks)  -- T scores / proj accum / B bcast
        #   po: 2 x [128,512]f32 (2 banks)  -- attn@V accum, both heads packed
        pp_n = ctx.enter_context(tc.tile_pool(name="pp_n", bufs=2, space="PSUM"))
        pp_t = ctx.enter_context(tc.tile_pool(name="pp_t", bufs=2, space="PSUM"))
        pp_o = ctx.enter_context(tc.tile_pool(name="pp_o", bufs=2, space="PSUM"))

        # ---- constants -------------------------------------------------
        w_k = consts.tile([128, KCH, HD], f16, name="w_k", tag="w_k")
        w_q = consts.tile([128, KCH, HD], f16, name="w_q", tag="w_q")
        nc.sync.dma_start(out=w_k, in_=wkT.rearrange("(ko p) m -> p ko m", p=128))
        nc.sync.dma_start(out=w_q, in_=wqT.rearrange("(ko p) m -> p ko m", p=128))
        bq_sb = consts.tile([HD, 1], f32, name="bq_sb", tag="bq_sb")
        bk_sb = consts.tile([HD, 1], f32, name="bk_sb", tag="bk_sb")
        nc.sync.dma_start(out=bk_sb, in_=bk[:, :])
        nc.sync.dma_start(out=bq_sb, in_=bq[:, :])
        w_v = consts.tile([128, KCH, HD], f16, name="w_v", tag="w_v")
        wo_sb = consts.tile([HD, D_OUT], f16, name="wo_sb", tag="wo_sb")
        ones_sb = consts.tile([128, 128], f16, name="ones_sb", tag="ones_sb")

        # per-head K-padded projections: other head's partitions are zero so
        # every score matmul runs with a full K=128 contraction (full-array
        # activity keeps the PE clock unthrottled; zeros contribute nothing)
        qhT_z = [consts.tile([128, BS], f16, name=f"qhT_z{h}", tag=f"qhT_z{h}")
                 for h in range(H_LOC)]
        khT_z = [consts.tile([128, BS], f16, name=f"khT_z{h}", tag=f"khT_z{h}")
                 for h in range(H_LOC)]
        nc.vector.memset(qhT_z[0][D_K:128, :], 0.0)
        nc.vector.memset(qhT_z[1][0:D_K, :], 0.0)
        nc.vector.memset(khT_z[0][D_K:128, :], 0.0)
        nc.vector.memset(khT_z[1][0:D_K, :], 0.0)
        # natural-layout v heads, both packed: cols 0-63 h0, 64-127 h1
        vhb = consts.tile([128, NSTRIP, HD], f16, name="vhb", tag="vhb")
        o_sb = consts.tile([HD, BS], f16, name="o_sb", tag="o_sb")

        # ---- k/q projections, q-block-major ----------------------------
        def proj_block(x_dram, nb, w_sb, b_sb, dst):
                slab = slabs.tile([128, KCH, QB], f16, name="slab", tag="slab")
                nc.sync.dma_start(out=slab, in_=x_dram[nb])
                ps = pp_t.tile([128, QB], f32, name="pj", tag="pt")
                for kc in range(KCH):
                    nc.tensor.matmul(
                        ps,
                        lhsT=w_sb[:, kc, :],
                        rhs=slab[:, kc, :],
                        start=(kc == 0),
                        stop=(kc == KCH - 1),
                    )
                with nc.allow_low_precision("fp16 projections feed fp16 matmul"):
                    nc.vector.tensor_scalar_add(
                        dst[0][0:D_K, nb * QB:(nb + 1) * QB],
                        ps[0:D_K, :], b_sb[0:D_K, 0:1],
                    )
                    nc.vector.tensor_scalar_add(
                        dst[1][D_K:128, nb * QB:(nb + 1) * QB],
                        ps[D_K:128, :], b_sb[D_K:128, 0:1],
                    )

        proj_block(kT, 0, w_k, bk_sb, khT_z)
        proj_block(kT, 1, w_k, bk_sb, khT_z)
        proj_block(qT, 0, w_q, bq_sb, qhT_z)
        proj_block(kT, 2, w_k, bk_sb, khT_z)
        proj_block(kT, 3, w_k, bk_sb, khT_z)

        # ---- vT slabs DMA'd now; vh blocks computed inside the unit loop
        nc.sync.dma_start(out=w_v, in_=wvT.rearrange("(ko p) m -> p ko m", p=128))
        vslabs = []
        for nb in range(4):
            vs = vspool.tile([128, KCH, QB], f16, name="vslab", tag="vslab")
            nc.sync.dma_start(out=vs, in_=vT[nb])
            vslabs.append(vs)
        nc.sync.dma_start(out=wo_sb, in_=woT[:, :])
        nc.sync.dma_start(out=ones_sb, in_=onesd[:, :])

        def vh_block(nb):
            vs = vslabs[nb]
            psv = pp_o.tile([128, 4, 128], f32, name="psv", tag="po")
            for kc in range(KCH):
                for bx in range(4):
                    nc.tensor.matmul(
                        psv[:, bx, :],
                        lhsT=vs[:, kc, bx * 128:(bx + 1) * 128],
                        rhs=w_v[:, kc, :],
                        start=(kc == 0 and bx % 4 == 0),
                        stop=(kc == KCH - 1),
                        skip_group_check=True,
                    )
            for bx in range(4):
                bc = nb * 4 + bx
                with nc.allow_low_precision("fp16 v-heads feed fp16 matmul"):
                    nc.vector.tensor_copy(
                        out=vhb[:, bc, :],
                        in_=psv[:, bx, :],
                    )

        # ---- attention: 32 units, both heads interleaved ----------------
        def finalize_o(qb, po, rrow0, rrow1):
            """Copy O' (both heads) out of PSUM, normalize by per-head 1/s
            rows broadcast over each head's partition range."""
            o_raw = orawpool.tile([128, QB], f32, name="o_raw", tag="o_raw")
            nc.vector.tensor_copy(out=o_raw, in_=po)
            pb = pp_t.tile([128, QB], f32, name="pb", tag="pt")
            nc.tensor.matmul(
                pb[0:D_K, :], lhsT=ones_sb[0:1, 0:D_K], rhs=rrow0[0:1, :],
                start=True, stop=True, skip_group_check=True,
            )
            nc.tensor.matmul(
                pb[D_K:HD, :], lhsT=ones_sb[0:1, 0:D_K], rhs=rrow1[0:1, :],
                start=True, stop=True, skip_group_check=True,
            )
            with nc.allow_low_precision("O output feeds fp16 out-projection"):
                nc.vector.tensor_mul(
                    o_sb[:, qb * QB:(qb + 1) * QB], o_raw, pb,
                )


        pending = []
        tick = 0
        po_tiles = {}
        rrows = {}
        hold = {}
        for qb in range(NQB):
            if qb >= 1:
                proj_block(qT, qb, w_q, bq_sb, qhT_z)
            for un in range(8):
                if qb == 0 and un % 2 == 0:
                    vh_block(un // 2)
                while pending and pending[0][0] <= tick:
                    finalize_o(*pending.pop(0)[1])
                tick += 1
                ms, khalf = qb * 4 + un // 2, un % 2
                k0 = khalf * 1024
                if un == 0:
                    for hh in range(H_LOC):
                        rrows[(qb, hh)] = rrowpool.tile(
                            [1, QB], f16, name="rrow", tag="rrow")
                # ---- N half-strips, both heads --------------------------
                pn_t = [pp_n.tile([128, 1024], f32, name="pn", tag="pn")
                        for _ in range(H_LOC)]
                for nb in range(2):
                    for h in range(H_LOC):
                        nc.tensor.matmul(
                            pn_t[h][:, nb * 512:(nb + 1) * 512],
                            lhsT=qhT_z[h][:, ms * 128:(ms + 1) * 128],
                            rhs=khT_z[h][:, k0 + nb * 512:k0 + (nb + 1) * 512],
                            start=True,
                            stop=True,
                        )
                for h in range(H_LOC):
                    u = upool.tile([128, 1024], f32, name="u", tag="u")
                    sp = spool.tile([128, 1], f32, name="sp", tag=f"sp{khalf}{h}")
                    nc.scalar.activation(
                        out=u, in_=pn_t[h], func=AF.Exp, scale=float(SCALE),
                        accum_out=sp,
                    )
                    if khalf == 0:
                        hold[h] = (u, sp)
                    else:
                        u0, s0 = hold[h]
                        s = spool.tile([128, 1], f32, name="s", tag=f"s{h}")
                        nc.vector.tensor_add(s, s0, sp)
                        r = spool.tile([128, 1], f32, name="r", tag=f"r{h}")
                        nc.vector.reciprocal(out=r, in_=s)
                        nc.gpsimd.dma_start(
                            out=rrows[(qb, h)][0:1,
                                               (ms % 4) * 128:(ms % 4 + 1) * 128],
                            in_=r[:, 0:1],
                        )
                        for uu, kh in ((u0, 0), (u, 1)):
                            a = apool.tile([128, 1024], f32, name="a", tag="a")
                            nc.vector.tensor_scalar_mul(a, uu, r[:, 0:1])
                            nc.sync.dma_start(
                                out=attn_out[h, ms * 128:(ms + 1) * 128,
                                             kh * 1024:(kh + 1) * 1024],
                                in_=a,
                            )
                # ---- two phase-T mini-strips (q-quarter qb) -------------
                q0 = qb * QB
                for ks in (2 * un, 2 * un + 1):
                    if ks == 0:
                        po_tiles[qb] = pp_o.tile(
                            [128, QB], f32, name="po", tag="po")
                    po = po_tiles[qb]
                    pt_t = [pp_t.tile([128, QB], f32, name="pt", tag="pt")
                            for _ in range(H_LOC)]
                    for hh in range(H_LOC):
                        nc.tensor.matmul(
                            pt_t[hh],
                            lhsT=khT_z[hh][:, ks * 128:(ks + 1) * 128],
                            rhs=qhT_z[hh][:, q0:q0 + QB],
                            start=True,
                            stop=True,
                        )
                    uts = []
                    for hh in range(H_LOC):
                        ut = utpool.tile([128, QB], f16, name="ut", tag="ut")
                        nc.scalar.activation(
                            out=ut, in_=pt_t[hh], func=AF.Exp, scale=float(SCALE))
                        uts.append(ut)
                    for hh in range(H_LOC):
                        nc.tensor.matmul(
                            po[hh * D_K:(hh + 1) * D_K, :],
                            lhsT=vhb[:, ks, hh * D_K:(hh + 1) * D_K],
                            rhs=uts[hh],
                            start=(ks == 0),
                            stop=(ks == NSTRIP - 1),
                            skip_group_check=True,
                        )
                    if ks == NSTRIP - 1:
                        pending.append(
                            (tick + (6 if qb < NQB - 1 else 0),
                             (qb, po, rrows[(qb, 0)], rrows[(qb, 1)])))
        def outproj(bc):
            pout = pp_n.tile([128, 1024], f32, name="pout", tag="pn")
            for oc in range(2):
                nc.tensor.matmul(
                    pout[:, oc * 512:(oc + 1) * 512],
                    lhsT=o_sb[:, bc * 128:(bc + 1) * 128],
                    rhs=wo_sb[:, oc * 512:(oc + 1) * 512],
                    start=True,
                    stop=True,
                )
            osb = outpool.tile([128, 1024], f32, name="osb", tag="osb")
            if bc % 2 == 0:
                nc.vector.tensor_copy(out=osb, in_=pout)
            else:
                nc.scalar.copy(out=osb, in_=pout)
            nc.sync.dma_start(out=outp[bc * 128:(bc + 1) * 128, :], in_=osb)

        # quarters 0-2 are finalized already; emit their projection first so
        # it overlaps the last quarter's finalize chain
        n_ready = (NQB - len(pending)) * 4
        for bc in range(n_ready):
            outproj(bc)
        while pending:
            finalize_o(*pending.pop(0)[1])
        for bc in range(n_ready, NSTRIP):
            outproj(bc)

    nc.compile()
    return nc


def _get_nc():
    if "nc" not in _CACHE:
        _CACHE["nc"] = _build_bass()
    return _CACHE["nc"]


def _make_in_maps(q, k, v, Wq, bq, Wk, bk, Wv, Wo):
    def _blocked(x):
        # [BS, D_IN] -> [4, 128, 8, 512]: slab nb holds x^T chunk
        # [p, ko, n] = x[nb*512+n, ko*128+p]
        return np.ascontiguousarray(
            x.astype(np.float16).reshape(4, 512, KCH, 128).transpose(0, 3, 2, 1))

    qT = _blocked(q)
    kT = _blocked(k)
    vT = _blocked(v)
    in_maps = []
    for c in range(N_CORES):
        sl = slice(c * HD, (c + 1) * HD)
        in_maps.append({
            "qT": qT,
            "kT": kT,
            "vT": vT,
            "wqT": np.ascontiguousarray(Wq[sl, :].T.astype(np.float16)),
            "wkT": np.ascontiguousarray(Wk[sl, :].T.astype(np.float16)),
            "wvT": np.ascontiguousarray(Wv[sl, :].T.astype(np.float16)),
            "woT": np.ascontiguousarray(Wo[:, c * HD:(c + 1) * HD].T.astype(np.float16)),
            "ones": np.ones((128, 128), dtype=np.float16),
            "bq": np.ascontiguousarray(bq[sl].reshape(HD, 1)),
            "bk": np.ascontiguousarray(bk[sl].reshape(HD, 1)),
        })
    return in_maps


def kernel(q, k, v, Wq, bq, Wk, bk, Wv, bv, Wo, bo):
    global LAST_RESULTS
    from concourse.bass_utils import run_bass_kernel_spmd

    q = np.ascontiguousarray(np.asarray(q, dtype=np.float32))
    k = np.ascontiguousarray(np.asarray(k, dtype=np.float32))
    v = np.ascontiguousarray(np.asarray(v, dtype=np.float32))
    Wq = np.asarray(Wq, dtype=np.float32)
    Wk = np.asarray(Wk, dtype=np.float32)
    Wv = np.asarray(Wv, dtype=np.float32)
    Wo = np.asarray(Wo, dtype=np.float32)
    bq = np.asarray(bq, dtype=np.float32)
    bk = np.asarray(bk, dtype=np.float32)
    bv = np.asarray(bv, dtype=np.float32)
    bo = np.asarray(bo, dtype=np.float32)

    in_maps = _make_in_maps(q, k, v, Wq, bq, Wk, bk, Wv, Wo)

    nc = _get_nc()
    res = run_bass_kernel_spmd(
        nc, in_maps, core_ids=list(range(N_CORES)),
    )
    LAST_RESULTS = res

    attn = np.concatenate([res.results[c]["attn"] for c in range(N_CORES)], axis=0)
    out = np.zeros((BS, D_OUT), dtype=np.float64)
    for c in range(N_CORES):
        out += res.results[c]["outp"]
    # bv folds through softmax (rows sum to 1) into a constant: Wo @ bv + bo
    out += (Wo.astype(np.float64) @ bv.astype(np.float64)) + bo.astype(np.float64)
    return out.astype(np.float32), attn


# revision 27
# speedup vs baseline: 1.0108x; 1.0108x over previous
"""Trainium2 Bass kernel for multi-head attention (BS=2048, D=1024, H=16, d_k=64).

Returns (output [2048,1024], attn [16,2048,2048]) like the reference.

Sharding: tensor-parallel over heads -- each of the 8 cores owns 2 heads.
Each core reads the full (host-pretransposed) q/k/v plus its head-slices of
the weights, computes its 2 heads' attention + attn output, writes its slice
of `attn` and a partial output projection.  Host sums the 8 partials and adds
the bias constants (bo + Wo@bv, which factor out exactly).

Per-core dataflow (all matmuls in float32r = full PE speed):
  - qhT/khT [128(head dims),2048] = W @ x^T projections (PSUM accum over 8
    k-chunks of D_IN, bias added on ScalarE eviction).
  - vh in natural [k-row, d] layout (lhsT = vT chunks), with a ones column
    appended so attn@V also produces softmax row sums.
  - phase N (per head, 16 q-strips): S = qhT^T @ khT -> PSUM [128,2048];
    ScalarE exp(0.125*S) with fused accum_out row-sums; VectorE reciprocal +
    tensor_scalar (per-partition) normalize; DMA the finished attn strip out.
  - phase T (per head, 2 q-halves, 16 k-strips): S^T = khT^T @ qhT; exp;
    attn@V accumulates O' [65,1024] over k-strips (row 64 = row sums);
    broadcast 1/s via a K=1 ones matmul, normalize O' on eviction.
  - output projection: out_part = O^T(both heads) @ WoT, accumulated as two
    K=64 matmuls per tile, evicted + DMA'd.
"""

import os
import sys
from contextlib import ExitStack

if "/opt/trn_rl_repo" not in sys.path:
    sys.path.insert(0, "/opt/trn_rl_repo")

import numpy as np

BS = 2048
D_IN = 1024
D_OUT = 1024
H = 16
D_K = 64
N_CORES = 8
H_LOC = H // N_CORES          # 2 heads per core
HD = H_LOC * D_K              # 128 head dims per core
KCH = D_IN // 128             # 8 contraction chunks for projections
NSTRIP = BS // 128            # 16 strips of 128
SCALE = 1.0 / np.sqrt(D_K)    # 0.125

_CACHE = {}

# Filled by the last run (for test.py): bass_utils.BassKernelResults
LAST_RESULTS = None


def _build_bass():
    import concourse.bass as bass
    import concourse.tile as tile
    import concourse.mybir as mybir
    from concourse import bacc

    f32 = mybir.dt.float32
    f16 = mybir.dt.float16
    AF = mybir.ActivationFunctionType

    nc = bacc.Bacc(None, target_bir_lowering=False)

    qT = nc.dram_tensor("qT", [4, 128, KCH, 512], f16, kind="ExternalInput")
    kT = nc.dram_tensor("kT", [4, 128, KCH, 512], f16, kind="ExternalInput")
    vT = nc.dram_tensor("vT", [4, 128, KCH, 512], f16, kind="ExternalInput")
    wqT = nc.dram_tensor("wqT", [D_IN, HD], f16, kind="ExternalInput")
    wkT = nc.dram_tensor("wkT", [D_IN, HD], f16, kind="ExternalInput")
    wvT = nc.dram_tensor("wvT", [D_IN, HD], f16, kind="ExternalInput")
    woT = nc.dram_tensor("woT", [HD, D_OUT], f16, kind="ExternalInput")
    onesd = nc.dram_tensor("ones", [128, 128], f16, kind="ExternalInput")
    bq = nc.dram_tensor("bq", [HD, 1], f32, kind="ExternalInput")
    bk = nc.dram_tensor("bk", [HD, 1], f32, kind="ExternalInput")

    attn_out = nc.dram_tensor("attn", [H_LOC, BS, BS], f32, kind="ExternalOutput")
    outp = nc.dram_tensor("outp", [BS, D_OUT], f32, kind="ExternalOutput")

    QB = 512                     # q-quarter width for the T stream
    NQB = BS // QB               # 4 quarters

    with tile.TileContext(nc) as tc, ExitStack() as ctx:
        consts = ctx.enter_context(tc.tile_pool(name="consts", bufs=1))
        slabs = ctx.enter_context(tc.tile_pool(name="slabs", bufs=3))
        vspool = ctx.enter_context(tc.tile_pool(name="vslabs", bufs=4))
        upool = ctx.enter_context(tc.tile_pool(name="u", bufs=6))
        utpool = ctx.enter_context(tc.tile_pool(name="ut", bufs=10))
        apool = ctx.enter_context(tc.tile_pool(name="a", bufs=4))
        spool = ctx.enter_context(tc.tile_pool(name="s", bufs=8))
        outpool = ctx.enter_context(tc.tile_pool(name="outsb", bufs=3))
        orawpool = ctx.enter_context(tc.tile_pool(name="oraw", bufs=3))
        rrowpool = ctx.enter_context(tc.tile_pool(name="rrow", bufs=6))

        # 8-bank PSUM budget:
        #   pn: 2 x [128,1024]f32 (4 banks) -- N scores (one slot per head),
        #       also vh-psum [128,8,128] and outproj [128,1024]
        #   pt: 2 x [128,512]f32 (2 banks)  -- T scores / proj accum / B bcast
        #   po: 2 x [128,512]f32 (2 banks)  -- attn@V accum, both heads packed
        pp_n = ctx.enter_context(tc.tile_pool(name="pp_n", bufs=2, space="PSUM"))
        pp_t = ctx.enter_context(tc.tile_pool(name="pp_t", bufs=2, space="PSUM"))
        pp_o = ctx.enter_context(tc.tile_pool(name="pp_o", bufs=2, space="PSUM"))

        # ---- constants -------------------------------------------------
        w_k = consts.tile([128, KCH, HD], f16, name="w_k", tag="w_k")
        w_q = consts.tile([128, KCH, HD], f16, name="w_q", tag="w_q")
        nc.sync.dma_start(out=w_k, in_=wkT.rearrange("(ko p) m -> p ko m", p=128))
        nc.sync.dma_start(out=w_q, in_=wqT.rearrange("(ko p) m -> p ko m", p=128))
        bq_sb = consts.tile([HD, 1], f32, name="bq_sb", tag="bq_sb")
        bk_sb = consts.tile([HD, 1], f32, name="bk_sb", tag="bk_sb")
        nc.sync.dma_start(out=bk_sb, in_=bk[:, :])
        nc.sync.dma_start(out=bq_sb, in_=bq[:, :])
        w_v = consts.tile([128, KCH, HD], f16, name="w_v", tag="w_v")
        wo_sb = consts.tile([HD, D_OUT], f16, name="wo_sb", tag="wo_sb")
        ones_sb = consts.tile([128, 128], f16, name="ones_sb", tag="ones_sb")

        # per-head K-padded projections: other head's partitions are zero so
        # every score matmul runs with a full K=128 contraction (full-array
        # activity keeps the PE clock unthrottled; zeros contribute nothing)
        qhT_z = [consts.tile([128, BS], f16, name=f"qhT_z{h}", tag=f"qhT_z{h}")
                 for h in range(H_LOC)]
        khT_z = [consts.tile([128, BS], f16, name=f"khT_z{h}", tag=f"khT_z{h}")
                 for h in range(H_LOC)]
        nc.vector.memset(qhT_z[0][D_K:128, :], 0.0)
        nc.vector.memset(qhT_z[1][0:D_K, :], 0.0)
        nc.vector.memset(khT_z[0][D_K:128, :], 0.0)
        nc.vector.memset(khT_z[1][0:D_K, :], 0.0)
        # natural-layout v heads, both packed: cols 0-63 h0, 64-127 h1
        vhb = consts.tile([128, NSTRIP, HD], f16, name="vhb", tag="vhb")
        o_sb = consts.tile([HD, BS], f16, name="o_sb", tag="o_sb")

        # ---- k/q projections, q-block-major ----------------------------
        def proj_block(x_dram, nb, w_sb, b_sb, dst):
                slab = slabs.tile([128, KCH, QB], f16, name="slab", tag="slab")
                nc.sync.dma_start(out=slab, in_=x_dram[nb])
                ps = pp_t.tile([128, QB], f32, name="pj", tag="pt")
                for kc in range(KCH):
                    nc.tensor.matmul(
                        ps,
                        lhsT=w_sb[:, kc, :],
                        rhs=slab[:, kc, :],
                        start=(kc == 0),
                        stop=(kc == KCH - 1),
                    )
                with nc.allow_low_precision("fp16 projections feed fp16 matmul"):
                    nc.vector.tensor_scalar_add(
                        dst[0][0:D_K, nb * QB:(nb + 1) * QB],
                        ps[0:D_K, :], b_sb[0:D_K, 0:1],
                    )
                    nc.vector.tensor_scalar_add(
                        dst[1][D_K:128, nb * QB:(nb + 1) * QB],
                        ps[D_K:128, :], b_sb[D_K:128, 0:1],
                    )

        proj_block(kT, 0, w_k, bk_sb, khT_z)
        proj_block(kT, 1, w_k, bk_sb, khT_z)
        proj_block(qT, 0, w_q, bq_sb, qhT_z)
        proj_block(kT, 2, w_k, bk_sb, khT_z)
        proj_block(kT, 3, w_k, bk_sb, khT_z)

        # ---- vT slabs DMA'd now; vh blocks computed inside the unit loop
        nc.sync.dma_start(out=w_v, in_=wvT.rearrange("(ko p) m -> p ko m", p=128))
        vslabs = []
        for nb in range(4):
            vs = vspool.tile([128, KCH, QB], f16, name="vslab", tag="vslab")
            nc.sync.dma_start(out=vs, in_=vT[nb])
            vslabs.append(vs)
        nc.sync.dma_start(out=wo_sb, in_=woT[:, :])
        nc.sync.dma_start(out=ones_sb, in_=onesd[:, :])

        def vh_block(nb):
            vs = vslabs[nb]
            psv = pp_t.tile([128, 4, 128], f32, name="psv", tag="pt")
            for kc in range(KCH):
                for bx in range(4):
                    nc.tensor.matmul(
                        psv[:, bx, :],
                        lhsT=vs[:, kc, bx * 128:(bx + 1) * 128],
                        rhs=w_v[:, kc, :],
                        start=(kc == 0 and bx % 4 == 0),
                        stop=(kc == KCH - 1),
                        skip_group_check=True,
                    )
            for bx in range(4):
                bc = nb * 4 + bx
                with nc.allow_low_precision("fp16 v-heads feed fp16 matmul"):
                    nc.vector.tensor_copy(
                        out=vhb[:, bc, :],
                        in_=psv[:, bx, :],
                    )

        # ---- attention: 32 units, both heads interleaved ----------------
        def finalize_o(qb, po, rrow0, rrow1):
            """Copy O' (both heads) out of PSUM, normalize by per-head 1/s
            rows broadcast over each head's partition range."""
            o_raw = orawpool.tile([128, QB], f32, name="o_raw", tag="o_raw")
            nc.vector.tensor_copy(out=o_raw, in_=po)
            pb = pp_t.tile([128, QB], f32, name="pb", tag="pt")
            nc.tensor.matmul(
                pb[0:D_K, :], lhsT=ones_sb[0:1, 0:D_K], rhs=rrow0[0:1, :],
                start=True, stop=True, skip_group_check=True,
            )
            nc.tensor.matmul(
                pb[D_K:HD, :], lhsT=ones_sb[0:1, 0:D_K], rhs=rrow1[0:1, :],
                start=True, stop=True, skip_group_check=True,
            )
            with nc.allow_low_precision("O output feeds fp16 out-projection"):
                nc.vector.tensor_mul(
                    o_sb[:, qb * QB:(qb + 1) * QB], o_raw, pb,
                )


        pending = []
        tick = 0
        po_tiles = {}
        rrows = {}
        hold = {}
        for qb in range(NQB):
            if qb >= 1:
                proj_block(qT, qb, w_q, bq_sb, qhT_z)
            for un in range(8):
                if qb == 0 and un % 2 == 0:
                    vh_block(un // 2)
                while pending and pending[0][0] <= tick:
                    finalize_o(*pending.pop(0)[1])
                tick += 1
                ms, khalf = qb * 4 + un // 2, un % 2
                k0 = khalf * 1024
                if un == 0:
                    for hh in range(H_LOC):
                        rrows[(qb, hh)] = rrowpool.tile(
                            [1, QB], f16, name="rrow", tag="rrow")
                # ---- N half-strips, both heads --------------------------
                pn_t = [pp_n.tile([128, 1024], f32, name="pn", tag="pn")
                        for _ in range(H_LOC)]
                for nb in range(2):
                    for h in range(H_LOC):
                        nc.tensor.matmul(
                            pn_t[h][:, nb * 512:(nb + 1) * 512],
                            lhsT=qhT_z[h][:, ms * 128:(ms + 1) * 128],
                            rhs=khT_z[h][:, k0 + nb * 512:k0 + (nb + 1) * 512],
                            start=True,
                            stop=True,
                        )
                for h in range(H_LOC):
                    u = upool.tile([128, 1024], f32, name="u", tag="u")
                    sp = spool.tile([128, 1], f32, name="sp", tag=f"sp{khalf}{h}")
                    nc.scalar.activation(
                        out=u, in_=pn_t[h], func=AF.Exp, scale=float(SCALE),
                        accum_out=sp,
                    )
                    if khalf == 0:
                        hold[h] = (u, sp)
                    else:
                        u0, s0 = hold[h]
                        s = spool.tile([128, 1], f32, name="s", tag=f"s{h}")
                        nc.vector.tensor_add(s, s0, sp)
                        r = spool.tile([128, 1], f32, name="r", tag=f"r{h}")
                        nc.vector.reciprocal(out=r, in_=s)
                        nc.gpsimd.dma_start(
                            out=rrows[(qb, h)][0:1,
                                               (ms % 4) * 128:(ms % 4 + 1) * 128],
                            in_=r[:, 0:1],
                        )
                        for uu, kh in ((u0, 0), (u, 1)):
                            a = apool.tile([128, 1024], f32, name="a", tag="a")
                            nc.vector.tensor_scalar_mul(a, uu, r[:, 0:1])
                            nc.sync.dma_start(
                                out=attn_out[h, ms * 128:(ms + 1) * 128,
                                             kh * 1024:(kh + 1) * 1024],
                                in_=a,
                            )
                # ---- two phase-T mini-strips (q-quarter qb) -------------
                q0 = qb * QB
                for ks in (2 * un, 2 * un + 1):
                    if ks == 0:
                        po_tiles[qb] = pp_o.tile(
                            [128, QB], f32, name="po", tag="po")
                    po = po_tiles[qb]
                    pt_t = [pp_t.tile([128, QB], f32, name="pt", tag="pt")
                            for _ in range(H_LOC)]
                    for hh in range(H_LOC):
                        nc.tensor.matmul(
                            pt_t[hh],
                            lhsT=khT_z[hh][:, ks * 128:(ks + 1) * 128],
                            rhs=qhT_z[hh][:, q0:q0 + QB],
                            start=True,
                            stop=True,
                        )
                    uts = []
                    for hh in range(H_LOC):
                        ut = utpool.tile([128, QB], f16, name="ut", tag="ut")
                        nc.scalar.activation(
                            out=ut, in_=pt_t[hh], func=AF.Exp, scale=float(SCALE))
                        uts.append(ut)
                    for hh in range(H_LOC):
                        nc.tensor.matmul(
                            po[hh * D_K:(hh + 1) * D_K, :],
                            lhsT=vhb[:, ks, hh * D_K:(hh + 1) * D_K],
                            rhs=uts[hh],
                            start=(ks == 0),
                            stop=(ks == NSTRIP - 1),
                            skip_group_check=True,
                        )
                    if ks == NSTRIP - 1:
                        pending.append(
                            (tick + (6 if qb < NQB - 1 else 0),
                             (qb, po, rrows[(qb, 0)], rrows[(qb, 1)])))
        def outproj(bc):
            pout = pp_n.tile([128, 1024], f32, name="pout", tag="pn")
            for oc in range(2):
                nc.tensor.matmul(
                    pout[:, oc * 512:(oc + 1) * 512],
                    lhsT=o_sb[:, bc * 128:(bc + 1) * 128],
                    rhs=wo_sb[:, oc * 512:(oc + 1) * 512],
                    start=True,
                    stop=True,
                )
            osb = outpool.tile([128, 1024], f32, name="osb", tag="osb")
            if bc % 2 == 0:
                nc.vector.tensor_copy(out=osb, in_=pout)
            else:
                nc.scalar.copy(out=osb, in_=pout)
            nc.sync.dma_start(out=outp[bc * 128:(bc + 1) * 128, :], in_=osb)

        # quarters 0-2 are finalized already; emit their projection first so
        # it overlaps the last quarter's finalize chain
        n_ready = (NQB - len(pending)) * 4
        for bc in range(n_ready):
            outproj(bc)
        while pending:
            finalize_o(*pending.pop(0)[1])
        for bc in range(n_ready, NSTRIP):
            outproj(bc)

    nc.compile()
    return nc


def _get_nc():
    if "nc" not in _CACHE:
        _CACHE["nc"] = _build_bass()
    return _CACHE["nc"]


def _make_in_maps(q, k, v, Wq, bq, Wk, bk, Wv, Wo):
    def _blocked(x):
        # [BS, D_IN] -> [4, 128, 8, 512]: slab nb holds x^T chunk
        # [p, ko, n] = x[nb*512+n, ko*128+p]
        return np.ascontiguousarray(
            x.astype(np.float16).reshape(4, 512, KCH, 128).transpose(0, 3, 2, 1))

    qT = _blocked(q)
    kT = _blocked(k)
    vT = _blocked(v)
    in_maps = []
    for c in range(N_CORES):
        sl = slice(c * HD, (c + 1) * HD)
        in_maps.append({
            "qT": qT,
            "kT": kT,
            "vT": vT,
            "wqT": np.ascontiguousarray(Wq[sl, :].T.astype(np.float16)),
            "wkT": np.ascontiguousarray(Wk[sl, :].T.astype(np.float16)),
            "wvT": np.ascontiguousarray(Wv[sl, :].T.astype(np.float16)),
            "woT": np.ascontiguousarray(Wo[:, c * HD:(c + 1) * HD].T.astype(np.float16)),
            "ones": np.ones((128, 128), dtype=np.float16),
            "bq": np.ascontiguousarray(bq[sl].reshape(HD, 1)),
            "bk": np.ascontiguousarray(bk[sl].reshape(HD, 1)),
        })
    return in_maps


def kernel(q, k, v, Wq, bq, Wk, bk, Wv, bv, Wo, bo):
    global LAST_RESULTS
    from concourse.bass_utils import run_bass_kernel_spmd

    q = np.ascontiguousarray(np.asarray(q, dtype=np.float32))
    k = np.ascontiguousarray(np.asarray(k, dtype=np.float32))
    v = np.ascontiguousarray(np.asarray(v, dtype=np.float32))
    Wq = np.asarray(Wq, dtype=np.float32)
    Wk = np.asarray(Wk, dtype=np.float32)
    Wv = np.asarray(Wv, dtype=np.float32)
    Wo = np.asarray(Wo, dtype=np.float32)
    bq = np.asarray(bq, dtype=np.float32)
    bk = np.asarray(bk, dtype=np.float32)
    bv = np.asarray(bv, dtype=np.float32)
    bo = np.asarray(bo, dtype=np.float32)

    in_maps = _make_in_maps(q, k, v, Wq, bq, Wk, bk, Wv, Wo)

    nc = _get_nc()
    res = run_bass_kernel_spmd(
        nc, in_maps, core_ids=list(range(N_CORES)),
    )
    LAST_RESULTS = res

    attn = np.concatenate([res.results[c]["attn"] for c in range(N_CORES)], axis=0)
    out = np.zeros((BS, D_OUT), dtype=np.float64)
    for c in range(N_CORES):
        out += res.results[c]["outp"]
    # bv folds through softmax (rows sum to 1) into a constant: Wo @ bv + bo
    out += (Wo.astype(np.float64) @ bv.astype(np.float64)) + bo.astype(np.float64)
    return out.astype(np.float32), attn


# revision 28
# speedup vs baseline: 1.0552x; 1.0439x over previous
"""Trainium2 Bass kernel for multi-head attention (BS=2048, D=1024, H=16, d_k=64).

Returns (output [2048,1024], attn [16,2048,2048]) like the reference.

Sharding: tensor-parallel over heads -- each of the 8 cores owns 2 heads.
Each core reads the full (host-pretransposed) q/k/v plus its head-slices of
the weights, computes its 2 heads' attention + attn output, writes its slice
of `attn` and a partial output projection.  Host sums the 8 partials and adds
the bias constants (bo + Wo@bv, which factor out exactly).

Per-core dataflow (all matmuls in float32r = full PE speed):
  - qhT/khT [128(head dims),2048] = W @ x^T projections (PSUM accum over 8
    k-chunks of D_IN, bias added on ScalarE eviction).
  - vh in natural [k-row, d] layout (lhsT = vT chunks), with a ones column
    appended so attn@V also produces softmax row sums.
  - phase N (per head, 16 q-strips): S = qhT^T @ khT -> PSUM [128,2048];
    ScalarE exp(0.125*S) with fused accum_out row-sums; VectorE reciprocal +
    tensor_scalar (per-partition) normalize; DMA the finished attn strip out.
  - phase T (per head, 2 q-halves, 16 k-strips): S^T = khT^T @ qhT; exp;
    attn@V accumulates O' [65,1024] over k-strips (row 64 = row sums);
    broadcast 1/s via a K=1 ones matmul, normalize O' on eviction.
  - output projection: out_part = O^T(both heads) @ WoT, accumulated as two
    K=64 matmuls per tile, evicted + DMA'd.
"""

import os
import sys
from contextlib import ExitStack

if "/opt/trn_rl_repo" not in sys.path:
    sys.path.insert(0, "/opt/trn_rl_repo")

import numpy as np

BS = 2048
D_IN = 1024
D_OUT = 1024
H = 16
D_K = 64
N_CORES = 8
H_LOC = H // N_CORES          # 2 heads per core
HD = H_LOC * D_K              # 128 head dims per core
KCH = D_IN // 128             # 8 contraction chunks for projections
NSTRIP = BS // 128            # 16 strips of 128
SCALE = 1.0 / np.sqrt(D_K)    # 0.125

_CACHE = {}

# Filled by the last run (for test.py): bass_utils.BassKernelResults
LAST_RESULTS = None


def _build_bass():
    import concourse.bass as bass
    import concourse.tile as tile
    import concourse.mybir as mybir
    from concourse import bacc

    f32 = mybir.dt.float32
    f16 = mybir.dt.float16
    AF = mybir.ActivationFunctionType

    nc = bacc.Bacc(None, target_bir_lowering=False)

    qT = nc.dram_tensor("qT", [4, 128, KCH, 512], f16, kind="ExternalInput")
    kT = nc.dram_tensor("kT", [4, 128, KCH, 512], f16, kind="ExternalInput")
    vT = nc.dram_tensor("vT", [4, 128, KCH, 512], f16, kind="ExternalInput")
    wqT = nc.dram_tensor("wqT", [D_IN, HD], f16, kind="ExternalInput")
    wkT = nc.dram_tensor("wkT", [D_IN, HD], f16, kind="ExternalInput")
    wvT = nc.dram_tensor("wvT", [D_IN, HD], f16, kind="ExternalInput")
    woT = nc.dram_tensor("woT", [HD, D_OUT], f16, kind="ExternalInput")
    onesd = nc.dram_tensor("ones", [128, 128], f16, kind="ExternalInput")
    bq = nc.dram_tensor("bq", [HD, 1], f32, kind="ExternalInput")
    bk = nc.dram_tensor("bk", [HD, 1], f32, kind="ExternalInput")

    attn_out = nc.dram_tensor("attn", [H_LOC, BS, BS], f32, kind="ExternalOutput")
    outp = nc.dram_tensor("outp", [BS, D_OUT], f32, kind="ExternalOutput")

    QB = 512                     # q-quarter width for the T stream
    NQB = BS // QB               # 4 quarters

    with tile.TileContext(nc) as tc, ExitStack() as ctx:
        consts = ctx.enter_context(tc.tile_pool(name="consts", bufs=1))
        slabs = ctx.enter_context(tc.tile_pool(name="slabs", bufs=3))
        vspool = ctx.enter_context(tc.tile_pool(name="vslabs", bufs=4))
        upool = ctx.enter_context(tc.tile_pool(name="u", bufs=6))
        utpool = ctx.enter_context(tc.tile_pool(name="ut", bufs=10))
        apool = ctx.enter_context(tc.tile_pool(name="a", bufs=4))
        spool = ctx.enter_context(tc.tile_pool(name="s", bufs=8))
        outpool = ctx.enter_context(tc.tile_pool(name="outsb", bufs=3))
        orawpool = ctx.enter_context(tc.tile_pool(name="oraw", bufs=3))
        rrowpool = ctx.enter_context(tc.tile_pool(name="rrow", bufs=6))

        # 8-bank PSUM budget:
        #   pn: 2 x [128,1024]f32 (4 banks) -- N scores (one slot per head),
        #       also vh-psum [128,8,128] and outproj [128,1024]
        #   pt: 2 x [128,512]f32 (2 banks)  -- T scores / proj accum / B bcast
        #   po: 2 x [128,512]f32 (2 banks)  -- attn@V accum, both heads packed
        pp_n = ctx.enter_context(tc.tile_pool(name="pp_n", bufs=2, space="PSUM"))
        pp_t = ctx.enter_context(tc.tile_pool(name="pp_t", bufs=2, space="PSUM"))
        pp_o = ctx.enter_context(tc.tile_pool(name="pp_o", bufs=2, space="PSUM"))

        # ---- constants -------------------------------------------------
        w_k = consts.tile([128, KCH, HD], f16, name="w_k", tag="w_k")
        w_q = consts.tile([128, KCH, HD], f16, name="w_q", tag="w_q")
        nc.sync.dma_start(out=w_k, in_=wkT.rearrange("(ko p) m -> p ko m", p=128))
        nc.sync.dma_start(out=w_q, in_=wqT.rearrange("(ko p) m -> p ko m", p=128))
        bq_sb = consts.tile([HD, 1], f32, name="bq_sb", tag="bq_sb")
        bk_sb = consts.tile([HD, 1], f32, name="bk_sb", tag="bk_sb")
        nc.sync.dma_start(out=bk_sb, in_=bk[:, :])
        nc.sync.dma_start(out=bq_sb, in_=bq[:, :])
        w_v = consts.tile([128, KCH, HD], f16, name="w_v", tag="w_v")
        wo_sb = consts.tile([HD, D_OUT], f16, name="wo_sb", tag="wo_sb")
        ones_sb = consts.tile([128, 128], f16, name="ones_sb", tag="ones_sb")

        # per-head K-padded projections: other head's partitions are zero so
        # every score matmul runs with a full K=128 contraction (full-array
        # activity keeps the PE clock unthrottled; zeros contribute nothing)
        qhT_z = [consts.tile([128, BS], f16, name=f"qhT_z{h}", tag=f"qhT_z{h}")
                 for h in range(H_LOC)]
        khT_z = [consts.tile([128, BS], f16, name=f"khT_z{h}", tag=f"khT_z{h}")
                 for h in range(H_LOC)]
        nc.vector.memset(qhT_z[0][D_K:128, :], 0.0)
        nc.vector.memset(qhT_z[1][0:D_K, :], 0.0)
        nc.vector.memset(khT_z[0][D_K:128, :], 0.0)
        nc.vector.memset(khT_z[1][0:D_K, :], 0.0)
        # natural-layout v heads, both packed: cols 0-63 h0, 64-127 h1
        vhb = consts.tile([128, NSTRIP, HD], f16, name="vhb", tag="vhb")
        o_sb = consts.tile([HD, BS], f16, name="o_sb", tag="o_sb")

        # ---- k/q projections, q-block-major ----------------------------
        def proj_block(x_dram, nb, w_sb, b_sb, dst):
                slab = slabs.tile([128, KCH, QB], f16, name="slab", tag="slab")
                nc.sync.dma_start(out=slab, in_=x_dram[nb])
                ps = pp_t.tile([128, QB], f32, name="pj", tag="pt")
                for kc in range(KCH):
                    nc.tensor.matmul(
                        ps,
                        lhsT=w_sb[:, kc, :],
                        rhs=slab[:, kc, :],
                        start=(kc == 0),
                        stop=(kc == KCH - 1),
                    )
                with nc.allow_low_precision("fp16 projections feed fp16 matmul"):
                    nc.vector.tensor_scalar_add(
                        dst[0][0:D_K, nb * QB:(nb + 1) * QB],
                        ps[0:D_K, :], b_sb[0:D_K, 0:1],
                    )
                    nc.vector.tensor_scalar_add(
                        dst[1][D_K:128, nb * QB:(nb + 1) * QB],
                        ps[D_K:128, :], b_sb[D_K:128, 0:1],
                    )

        proj_block(kT, 0, w_k, bk_sb, khT_z)
        proj_block(kT, 1, w_k, bk_sb, khT_z)
        proj_block(qT, 0, w_q, bq_sb, qhT_z)
        proj_block(kT, 2, w_k, bk_sb, khT_z)
        proj_block(kT, 3, w_k, bk_sb, khT_z)

        # ---- vT slabs DMA'd now; vh blocks computed inside the unit loop
        nc.sync.dma_start(out=w_v, in_=wvT.rearrange("(ko p) m -> p ko m", p=128))
        vslabs = []
        for nb in range(4):
            vs = vspool.tile([128, KCH, QB], f16, name="vslab", tag="vslab")
            nc.sync.dma_start(out=vs, in_=vT[nb])
            vslabs.append(vs)
        nc.sync.dma_start(out=wo_sb, in_=woT[:, :])
        nc.sync.dma_start(out=ones_sb, in_=onesd[:, :])

        def vh_block(nb):
            vs = vslabs[nb]
            psv = pp_t.tile([128, 4, 128], f32, name="psv", tag="pt")
            for kc in range(KCH):
                for bx in range(4):
                    nc.tensor.matmul(
                        psv[:, bx, :],
                        lhsT=vs[:, kc, bx * 128:(bx + 1) * 128],
                        rhs=w_v[:, kc, :],
                        start=(kc == 0 and bx % 4 == 0),
                        stop=(kc == KCH - 1),
                        skip_group_check=True,
                    )
            for bx in range(4):
                bc = nb * 4 + bx
                with nc.allow_low_precision("fp16 v-heads feed fp16 matmul"):
                    nc.vector.tensor_copy(
                        out=vhb[:, bc, :],
                        in_=psv[:, bx, :],
                    )

        # ---- attention: 32 units, both heads interleaved ----------------
        def finalize_o(qb, po, rrow0, rrow1):
            """Copy O' (both heads) out of PSUM, normalize by per-head 1/s
            rows broadcast over each head's partition range."""
            o_raw = orawpool.tile([128, QB], f32, name="o_raw", tag="o_raw")
            nc.vector.tensor_copy(out=o_raw, in_=po)
            pb = pp_t.tile([128, QB], f32, name="pb", tag="pt")
            nc.tensor.matmul(
                pb[0:D_K, :], lhsT=ones_sb[0:1, 0:D_K], rhs=rrow0[0:1, :],
                start=True, stop=True, skip_group_check=True,
            )
            nc.tensor.matmul(
                pb[D_K:HD, :], lhsT=ones_sb[0:1, 0:D_K], rhs=rrow1[0:1, :],
                start=True, stop=True, skip_group_check=True,
            )
            with nc.allow_low_precision("O output feeds fp16 out-projection"):
                nc.vector.tensor_mul(
                    o_sb[:, qb * QB:(qb + 1) * QB], o_raw, pb,
                )


        pending = []
        tick = 0
        po_tiles = {}
        rrows = {}
        hold = {}
        for nb in range(4):
            vh_block(nb)
        for nb in range(1, 4):
            proj_block(qT, nb, w_q, bq_sb, qhT_z)
        for qb in range(NQB):
            for un in range(8):
                while pending and pending[0][0] <= tick:
                    finalize_o(*pending.pop(0)[1])
                tick += 1
                ms, khalf = qb * 4 + un // 2, un % 2
                k0 = khalf * 1024
                if un == 0:
                    for hh in range(H_LOC):
                        rrows[(qb, hh)] = rrowpool.tile(
                            [1, QB], f16, name="rrow", tag="rrow")
                # ---- N half-strips, both heads --------------------------
                pn_t = [pp_n.tile([128, 1024], f32, name="pn", tag="pn")
                        for _ in range(H_LOC)]
                for nb in range(2):
                    for h in range(H_LOC):
                        nc.tensor.matmul(
                            pn_t[h][:, nb * 512:(nb + 1) * 512],
                            lhsT=qhT_z[h][:, ms * 128:(ms + 1) * 128],
                            rhs=khT_z[h][:, k0 + nb * 512:k0 + (nb + 1) * 512],
                            start=True,
                            stop=True,
                        )
                for h in range(H_LOC):
                    u = upool.tile([128, 1024], f32, name="u", tag="u")
                    sp = spool.tile([128, 1], f32, name="sp", tag=f"sp{khalf}{h}")
                    nc.scalar.activation(
                        out=u, in_=pn_t[h], func=AF.Exp, scale=float(SCALE),
                        accum_out=sp,
                    )
                    if khalf == 0:
                        hold[h] = (u, sp)
                    else:
                        u0, s0 = hold[h]
                        s = spool.tile([128, 1], f32, name="s", tag=f"s{h}")
                        nc.vector.tensor_add(s, s0, sp)
                        r = spool.tile([128, 1], f32, name="r", tag=f"r{h}")
                        nc.vector.reciprocal(out=r, in_=s)
                        nc.gpsimd.dma_start(
                            out=rrows[(qb, h)][0:1,
                                               (ms % 4) * 128:(ms % 4 + 1) * 128],
                            in_=r[:, 0:1],
                        )
                        for uu, kh in ((u0, 0), (u, 1)):
                            a = apool.tile([128, 1024], f32, name="a", tag="a")
                            nc.vector.tensor_scalar_mul(a, uu, r[:, 0:1])
                            nc.sync.dma_start(
                                out=attn_out[h, ms * 128:(ms + 1) * 128,
                                             kh * 1024:(kh + 1) * 1024],
                                in_=a,
                            )
                # ---- two phase-T mini-strips (q-quarter qb) -------------
                q0 = qb * QB
                for ks in (2 * un, 2 * un + 1):
                    if ks == 0:
                        po_tiles[qb] = pp_o.tile(
                            [128, QB], f32, name="po", tag="po")
                    po = po_tiles[qb]
                    pt_t = [pp_t.tile([128, QB], f32, name="pt", tag="pt")
                            for _ in range(H_LOC)]
                    for hh in range(H_LOC):
                        nc.tensor.matmul(
                            pt_t[hh],
                            lhsT=khT_z[hh][:, ks * 128:(ks + 1) * 128],
                            rhs=qhT_z[hh][:, q0:q0 + QB],
                            start=True,
                            stop=True,
                        )
                    uts = []
                    for hh in range(H_LOC):
                        ut = utpool.tile([128, QB], f16, name="ut", tag="ut")
                        nc.scalar.activation(
                            out=ut, in_=pt_t[hh], func=AF.Exp, scale=float(SCALE))
                        uts.append(ut)
                    for hh in range(H_LOC):
                        nc.tensor.matmul(
                            po[hh * D_K:(hh + 1) * D_K, :],
                            lhsT=vhb[:, ks, hh * D_K:(hh + 1) * D_K],
                            rhs=uts[hh],
                            start=(ks == 0),
                            stop=(ks == NSTRIP - 1),
                            skip_group_check=True,
                        )
                    if ks == NSTRIP - 1:
                        pending.append(
                            (tick + (6 if qb < NQB - 1 else 0),
                             (qb, po, rrows[(qb, 0)], rrows[(qb, 1)])))
        def outproj(bc):
            pout = pp_n.tile([128, 1024], f32, name="pout", tag="pn")
            for oc in range(2):
                nc.tensor.matmul(
                    pout[:, oc * 512:(oc + 1) * 512],
                    lhsT=o_sb[:, bc * 128:(bc + 1) * 128],
                    rhs=wo_sb[:, oc * 512:(oc + 1) * 512],
                    start=True,
                    stop=True,
                )
            osb = outpool.tile([128, 1024], f32, name="osb", tag="osb")
            if bc % 2 == 0:
                nc.vector.tensor_copy(out=osb, in_=pout)
            else:
                nc.scalar.copy(out=osb, in_=pout)
            nc.sync.dma_start(out=outp[bc * 128:(bc + 1) * 128, :], in_=osb)

        # quarters 0-2 are finalized already; emit their projection first so
        # it overlaps the last quarter's finalize chain
        n_ready = (NQB - len(pending)) * 4
        for bc in range(n_ready):
            outproj(bc)
        while pending:
            finalize_o(*pending.pop(0)[1])
        for bc in range(n_ready, NSTRIP):
            outproj(bc)

    nc.compile()
    return nc


def _get_nc():
    if "nc" not in _CACHE:
        _CACHE["nc"] = _build_bass()
    return _CACHE["nc"]


def _make_in_maps(q, k, v, Wq, bq, Wk, bk, Wv, Wo):
    def _blocked(x):
        # [BS, D_IN] -> [4, 128, 8, 512]: slab nb holds x^T chunk
        # [p, ko, n] = x[nb*512+n, ko*128+p]
        return np.ascontiguousarray(
            x.astype(np.float16).reshape(4, 512, KCH, 128).transpose(0, 3, 2, 1))

    qT = _blocked(q)
    kT = _blocked(k)
    vT = _blocked(v)
    in_maps = []
    for c in range(N_CORES):
        sl = slice(c * HD, (c + 1) * HD)
        in_maps.append({
            "qT": qT,
            "kT": kT,
            "vT": vT,
            "wqT": np.ascontiguousarray(Wq[sl, :].T.astype(np.float16)),
            "wkT": np.ascontiguousarray(Wk[sl, :].T.astype(np.float16)),
            "wvT": np.ascontiguousarray(Wv[sl, :].T.astype(np.float16)),
            "woT": np.ascontiguousarray(Wo[:, c * HD:(c + 1) * HD].T.astype(np.float16)),
            "ones": np.ones((128, 128), dtype=np.float16),
            "bq": np.ascontiguousarray(bq[sl].reshape(HD, 1)),
            "bk": np.ascontiguousarray(bk[sl].reshape(HD, 1)),
        })
    return in_maps


def kernel(q, k, v, Wq, bq, Wk, bk, Wv, bv, Wo, bo):
    global LAST_RESULTS
    from concourse.bass_utils import run_bass_kernel_spmd

    q = np.ascontiguousarray(np.asarray(q, dtype=np.float32))
    k = np.ascontiguousarray(np.asarray(k, dtype=np.float32))
    v = np.ascontiguousarray(np.asarray(v, dtype=np.float32))
    Wq = np.asarray(Wq, dtype=np.float32)
    Wk = np.asarray(Wk, dtype=np.float32)
    Wv = np.asarray(Wv, dtype=np.float32)
    Wo = np.asarray(Wo, dtype=np.float32)
    bq = np.asarray(bq, dtype=np.float32)
    bk = np.asarray(bk, dtype=np.float32)
    bv = np.asarray(bv, dtype=np.float32)
    bo = np.asarray(bo, dtype=np.float32)

    in_maps = _make_in_maps(q, k, v, Wq, bq, Wk, bk, Wv, Wo)

    nc = _get_nc()
    res = run_bass_kernel_spmd(
        nc, in_maps, core_ids=list(range(N_CORES)),
    )
    LAST_RESULTS = res

    attn = np.concatenate([res.results[c]["attn"] for c in range(N_CORES)], axis=0)
    out = np.zeros((BS, D_OUT), dtype=np.float64)
    for c in range(N_CORES):
        out += res.results[c]["outp"]
    # bv folds through softmax (rows sum to 1) into a constant: Wo @ bv + bo
    out += (Wo.astype(np.float64) @ bv.astype(np.float64)) + bo.astype(np.float64)
    return out.astype(np.float32), attn


# revision 29
# speedup vs baseline: 1.0698x; 1.0139x over previous
"""Trainium2 Bass kernel for multi-head attention (BS=2048, D=1024, H=16, d_k=64).

Returns (output [2048,1024], attn [16,2048,2048]) like the reference.

Sharding: tensor-parallel over heads -- each of the 8 cores owns 2 heads.
Each core reads the full (host-pretransposed) q/k/v plus its head-slices of
the weights, computes its 2 heads' attention + attn output, writes its slice
of `attn` and a partial output projection.  Host sums the 8 partials and adds
the bias constants (bo + Wo@bv, which factor out exactly).

Per-core dataflow (all matmuls in float32r = full PE speed):
  - qhT/khT [128(head dims),2048] = W @ x^T projections (PSUM accum over 8
    k-chunks of D_IN, bias added on ScalarE eviction).
  - vh in natural [k-row, d] layout (lhsT = vT chunks), with a ones column
    appended so attn@V also produces softmax row sums.
  - phase N (per head, 16 q-strips): S = qhT^T @ khT -> PSUM [128,2048];
    ScalarE exp(0.125*S) with fused accum_out row-sums; VectorE reciprocal +
    tensor_scalar (per-partition) normalize; DMA the finished attn strip out.
  - phase T (per head, 2 q-halves, 16 k-strips): S^T = khT^T @ qhT; exp;
    attn@V accumulates O' [65,1024] over k-strips (row 64 = row sums);
    broadcast 1/s via a K=1 ones matmul, normalize O' on eviction.
  - output projection: out_part = O^T(both heads) @ WoT, accumulated as two
    K=64 matmuls per tile, evicted + DMA'd.
"""

import os
import sys
from contextlib import ExitStack

if "/opt/trn_rl_repo" not in sys.path:
    sys.path.insert(0, "/opt/trn_rl_repo")

import numpy as np

BS = 2048
D_IN = 1024
D_OUT = 1024
H = 16
D_K = 64
N_CORES = 8
H_LOC = H // N_CORES          # 2 heads per core
HD = H_LOC * D_K              # 128 head dims per core
KCH = D_IN // 128             # 8 contraction chunks for projections
NSTRIP = BS // 128            # 16 strips of 128
SCALE = 1.0 / np.sqrt(D_K)    # 0.125

_CACHE = {}

# Filled by the last run (for test.py): bass_utils.BassKernelResults
LAST_RESULTS = None


def _build_bass():
    import concourse.bass as bass
    import concourse.tile as tile
    import concourse.mybir as mybir
    from concourse import bacc

    f32 = mybir.dt.float32
    f16 = mybir.dt.float16
    AF = mybir.ActivationFunctionType

    nc = bacc.Bacc(None, target_bir_lowering=False)

    qT = nc.dram_tensor("qT", [4, 128, KCH, 512], f16, kind="ExternalInput")
    kT = nc.dram_tensor("kT", [4, 128, KCH, 512], f16, kind="ExternalInput")
    vT = nc.dram_tensor("vT", [4, 128, KCH, 512], f16, kind="ExternalInput")
    wqT = nc.dram_tensor("wqT", [D_IN, HD], f16, kind="ExternalInput")
    wkT = nc.dram_tensor("wkT", [D_IN, HD], f16, kind="ExternalInput")
    wvT = nc.dram_tensor("wvT", [D_IN, HD], f16, kind="ExternalInput")
    woT = nc.dram_tensor("woT", [HD, D_OUT], f16, kind="ExternalInput")
    onesd = nc.dram_tensor("ones", [128, 128], f16, kind="ExternalInput")
    bq = nc.dram_tensor("bq", [HD, 1], f32, kind="ExternalInput")
    bk = nc.dram_tensor("bk", [HD, 1], f32, kind="ExternalInput")

    attn_out = nc.dram_tensor("attn", [H_LOC, BS, BS], f32, kind="ExternalOutput")
    outp = nc.dram_tensor("outp", [BS, D_OUT], f32, kind="ExternalOutput")

    QB = 512                     # q-quarter width for the T stream
    NQB = BS // QB               # 4 quarters

    with tile.TileContext(nc) as tc, ExitStack() as ctx:
        consts = ctx.enter_context(tc.tile_pool(name="consts", bufs=1))
        slabs = ctx.enter_context(tc.tile_pool(name="slabs", bufs=3))
        vspool = ctx.enter_context(tc.tile_pool(name="vslabs", bufs=4))
        upool = ctx.enter_context(tc.tile_pool(name="u", bufs=6))
        utpool = ctx.enter_context(tc.tile_pool(name="ut", bufs=10))
        apool = ctx.enter_context(tc.tile_pool(name="a", bufs=4))
        spool = ctx.enter_context(tc.tile_pool(name="s", bufs=8))
        outpool = ctx.enter_context(tc.tile_pool(name="outsb", bufs=3))
        orawpool = ctx.enter_context(tc.tile_pool(name="oraw", bufs=3))
        rrowpool = ctx.enter_context(tc.tile_pool(name="rrow", bufs=6))

        # 8-bank PSUM budget:
        #   pn: 2 x [128,1024]f32 (4 banks) -- N scores (one slot per head),
        #       also vh-psum [128,8,128] and outproj [128,1024]
        #   pt: 2 x [128,512]f32 (2 banks)  -- T scores / proj accum / B bcast
        #   po: 2 x [128,512]f32 (2 banks)  -- attn@V accum, both heads packed
        pp_n = ctx.enter_context(tc.tile_pool(name="pp_n", bufs=2, space="PSUM"))
        pp_t = ctx.enter_context(tc.tile_pool(name="pp_t", bufs=2, space="PSUM"))
        pp_o = ctx.enter_context(tc.tile_pool(name="pp_o", bufs=2, space="PSUM"))

        # ---- constants -------------------------------------------------
        w_k = consts.tile([128, KCH, HD], f16, name="w_k", tag="w_k")
        w_q = consts.tile([128, KCH, HD], f16, name="w_q", tag="w_q")
        nc.sync.dma_start(out=w_k, in_=wkT.rearrange("(ko p) m -> p ko m", p=128))
        nc.sync.dma_start(out=w_q, in_=wqT.rearrange("(ko p) m -> p ko m", p=128))
        bq_sb = consts.tile([HD, 1], f32, name="bq_sb", tag="bq_sb")
        bk_sb = consts.tile([HD, 1], f32, name="bk_sb", tag="bk_sb")
        nc.sync.dma_start(out=bk_sb, in_=bk[:, :])
        nc.sync.dma_start(out=bq_sb, in_=bq[:, :])
        w_v = consts.tile([128, KCH, HD], f16, name="w_v", tag="w_v")
        wo_sb = consts.tile([HD, D_OUT], f16, name="wo_sb", tag="wo_sb")
        ones_sb = consts.tile([128, 128], f16, name="ones_sb", tag="ones_sb")

        # per-head K-padded projections: other head's partitions are zero so
        # every score matmul runs with a full K=128 contraction (full-array
        # activity keeps the PE clock unthrottled; zeros contribute nothing)
        qhT_z = [consts.tile([128, BS], f16, name=f"qhT_z{h}", tag=f"qhT_z{h}")
                 for h in range(H_LOC)]
        khT_z = [consts.tile([128, BS], f16, name=f"khT_z{h}", tag=f"khT_z{h}")
                 for h in range(H_LOC)]
        nc.vector.memset(qhT_z[0][D_K:128, :], 0.0)
        nc.vector.memset(qhT_z[1][0:D_K, :], 0.0)
        nc.vector.memset(khT_z[0][D_K:128, :], 0.0)
        nc.vector.memset(khT_z[1][0:D_K, :], 0.0)
        # natural-layout v heads, both packed: cols 0-63 h0, 64-127 h1
        vhb = consts.tile([128, NSTRIP, HD], f16, name="vhb", tag="vhb")
        o_sb = consts.tile([HD, BS], f16, name="o_sb", tag="o_sb")

        # ---- k/q projections, q-block-major ----------------------------
        def proj_block(x_dram, nb, w_sb, b_sb, dst):
                slab = slabs.tile([128, KCH, QB], f16, name="slab", tag="slab")
                nc.sync.dma_start(out=slab, in_=x_dram[nb])
                ps = pp_t.tile([128, QB], f32, name="pj", tag="pt")
                for kc in range(KCH):
                    nc.tensor.matmul(
                        ps,
                        lhsT=w_sb[:, kc, :],
                        rhs=slab[:, kc, :],
                        start=(kc == 0),
                        stop=(kc == KCH - 1),
                    )
                with nc.allow_low_precision("fp16 projections feed fp16 matmul"):
                    nc.vector.tensor_scalar_add(
                        dst[0][0:D_K, nb * QB:(nb + 1) * QB],
                        ps[0:D_K, :], b_sb[0:D_K, 0:1],
                    )
                    nc.vector.tensor_scalar_add(
                        dst[1][D_K:128, nb * QB:(nb + 1) * QB],
                        ps[D_K:128, :], b_sb[D_K:128, 0:1],
                    )

        proj_block(kT, 0, w_k, bk_sb, khT_z)
        proj_block(kT, 1, w_k, bk_sb, khT_z)
        proj_block(qT, 0, w_q, bq_sb, qhT_z)
        proj_block(kT, 2, w_k, bk_sb, khT_z)
        proj_block(kT, 3, w_k, bk_sb, khT_z)

        # ---- vT slabs DMA'd now; vh blocks computed inside the unit loop
        nc.sync.dma_start(out=w_v, in_=wvT.rearrange("(ko p) m -> p ko m", p=128))
        vslabs = []
        for nb in range(4):
            vs = vspool.tile([128, KCH, QB], f16, name="vslab", tag="vslab")
            nc.sync.dma_start(out=vs, in_=vT[nb])
            vslabs.append(vs)
        nc.sync.dma_start(out=wo_sb, in_=woT[:, :])
        nc.sync.dma_start(out=ones_sb, in_=onesd[:, :])

        def vh_block(nb):
            vs = vslabs[nb]
            psv = pp_t.tile([128, 4, 128], f32, name="psv", tag="pt")
            for kc in range(KCH):
                for bx in range(4):
                    nc.tensor.matmul(
                        psv[:, bx, :],
                        lhsT=vs[:, kc, bx * 128:(bx + 1) * 128],
                        rhs=w_v[:, kc, :],
                        start=(kc == 0 and bx % 4 == 0),
                        stop=(kc == KCH - 1),
                        skip_group_check=True,
                    )
            for bx in range(4):
                bc = nb * 4 + bx
                with nc.allow_low_precision("fp16 v-heads feed fp16 matmul"):
                    nc.vector.tensor_copy(
                        out=vhb[:, bc, :],
                        in_=psv[:, bx, :],
                    )

        # ---- attention: 32 units, both heads interleaved ----------------
        def finalize_o(qb, po, rrow0, rrow1):
            """Copy O' (both heads) out of PSUM, normalize by per-head 1/s
            rows broadcast over each head's partition range."""
            o_raw = orawpool.tile([128, QB], f32, name="o_raw", tag="o_raw")
            nc.vector.tensor_copy(out=o_raw, in_=po)
            pb = pp_t.tile([128, QB], f32, name="pb", tag="pt")
            nc.tensor.matmul(
                pb[0:D_K, :], lhsT=ones_sb[0:1, 0:D_K], rhs=rrow0[0:1, :],
                start=True, stop=True, skip_group_check=True,
            )
            nc.tensor.matmul(
                pb[D_K:HD, :], lhsT=ones_sb[0:1, 0:D_K], rhs=rrow1[0:1, :],
                start=True, stop=True, skip_group_check=True,
            )
            with nc.allow_low_precision("O output feeds fp16 out-projection"):
                nc.vector.tensor_mul(
                    o_sb[:, qb * QB:(qb + 1) * QB], o_raw, pb,
                )


        pending = []
        tick = 0
        po_tiles = {}
        rrows = {}
        hold = {}
        for nb in range(4):
            vh_block(nb)
        for nb in range(1, 4):
            proj_block(qT, nb, w_q, bq_sb, qhT_z)
        for qb in range(NQB):
            for un in range(8):
                while pending and pending[0][0] <= tick:
                    finalize_o(*pending.pop(0)[1])
                tick += 1
                ms, khalf = qb * 4 + un // 2, un % 2
                k0 = khalf * 1024
                if un == 0:
                    for hh in range(H_LOC):
                        rrows[(qb, hh)] = rrowpool.tile(
                            [1, QB], f16, name="rrow", tag="rrow")
                # ---- N half-strips, both heads --------------------------
                pn_t = [pp_n.tile([128, 1024], f32, name="pn", tag="pn")
                        for _ in range(H_LOC)]
                for nb in range(2):
                    for h in range(H_LOC):
                        nc.tensor.matmul(
                            pn_t[h][:, nb * 512:(nb + 1) * 512],
                            lhsT=qhT_z[h][:, ms * 128:(ms + 1) * 128],
                            rhs=khT_z[h][:, k0 + nb * 512:k0 + (nb + 1) * 512],
                            start=True,
                            stop=True,
                        )
                for h in range(H_LOC):
                    u = upool.tile([128, 1024], f32, name="u", tag="u")
                    sp = spool.tile([128, 1], f32, name="sp", tag=f"sp{khalf}{h}")
                    nc.scalar.activation(
                        out=u, in_=pn_t[h], func=AF.Exp, scale=float(SCALE),
                        accum_out=sp,
                    )
                    if khalf == 0:
                        hold[h] = (u, sp)
                    else:
                        u0, s0 = hold[h]
                        s = spool.tile([128, 1], f32, name="s", tag=f"s{h}")
                        nc.vector.tensor_add(s, s0, sp)
                        r = spool.tile([128, 1], f32, name="r", tag=f"r{h}")
                        nc.vector.reciprocal(out=r, in_=s)
                        nc.gpsimd.dma_start(
                            out=rrows[(qb, h)][0:1,
                                               (ms % 4) * 128:(ms % 4 + 1) * 128],
                            in_=r[:, 0:1],
                        )
                        a = apool.tile([128, BS], f32, name="a", tag="a")
                        for uu, kh in ((u0, 0), (u, 1)):
                            nc.vector.tensor_scalar_mul(
                                a[:, kh * 1024:(kh + 1) * 1024], uu, r[:, 0:1])
                        nc.sync.dma_start(
                            out=attn_out[h, ms * 128:(ms + 1) * 128, :], in_=a,
                        )
                # ---- two phase-T mini-strips (q-quarter qb) -------------
                q0 = qb * QB
                for ks in (2 * un, 2 * un + 1):
                    if ks == 0:
                        po_tiles[qb] = pp_o.tile(
                            [128, QB], f32, name="po", tag="po")
                    po = po_tiles[qb]
                    pt_t = [pp_t.tile([128, QB], f32, name="pt", tag="pt")
                            for _ in range(H_LOC)]
                    for hh in range(H_LOC):
                        nc.tensor.matmul(
                            pt_t[hh],
                            lhsT=khT_z[hh][:, ks * 128:(ks + 1) * 128],
                            rhs=qhT_z[hh][:, q0:q0 + QB],
                            start=True,
                            stop=True,
                        )
                    uts = []
                    for hh in range(H_LOC):
                        ut = utpool.tile([128, QB], f16, name="ut", tag="ut")
                        nc.scalar.activation(
                            out=ut, in_=pt_t[hh], func=AF.Exp, scale=float(SCALE))
                        uts.append(ut)
                    for hh in range(H_LOC):
                        nc.tensor.matmul(
                            po[hh * D_K:(hh + 1) * D_K, :],
                            lhsT=vhb[:, ks, hh * D_K:(hh + 1) * D_K],
                            rhs=uts[hh],
                            start=(ks == 0),
                            stop=(ks == NSTRIP - 1),
                            skip_group_check=True,
                        )
                    if ks == NSTRIP - 1:
                        pending.append(
                            (tick + (6 if qb < NQB - 1 else 0),
                             (qb, po, rrows[(qb, 0)], rrows[(qb, 1)])))
        def outproj(bc):
            pout = pp_n.tile([128, 1024], f32, name="pout", tag="pn")
            for oc in range(2):
                nc.tensor.matmul(
                    pout[:, oc * 512:(oc + 1) * 512],
                    lhsT=o_sb[:, bc * 128:(bc + 1) * 128],
                    rhs=wo_sb[:, oc * 512:(oc + 1) * 512],
                    start=True,
                    stop=True,
                )
            osb = outpool.tile([128, 1024], f32, name="osb", tag="osb")
            if bc % 2 == 0:
                nc.vector.tensor_copy(out=osb, in_=pout)
            else:
                nc.scalar.copy(out=osb, in_=pout)
            nc.sync.dma_start(out=outp[bc * 128:(bc + 1) * 128, :], in_=osb)

        # quarters 0-2 are finalized already; emit their projection first so
        # it overlaps the last quarter's finalize chain
        n_ready = (NQB - len(pending)) * 4
        for bc in range(n_ready):
            outproj(bc)
        while pending:
            finalize_o(*pending.pop(0)[1])
        for bc in range(n_ready, NSTRIP):
            outproj(bc)

    nc.compile()
    return nc


def _get_nc():
    if "nc" not in _CACHE:
        _CACHE["nc"] = _build_bass()
    return _CACHE["nc"]


def _make_in_maps(q, k, v, Wq, bq, Wk, bk, Wv, Wo):
    def _blocked(x):
        # [BS, D_IN] -> [4, 128, 8, 512]: slab nb holds x^T chunk
        # [p, ko, n] = x[nb*512+n, ko*128+p]
        return np.ascontiguousarray(
            x.astype(np.float16).reshape(4, 512, KCH, 128).transpose(0, 3, 2, 1))

    qT = _blocked(q)
    kT = _blocked(k)
    vT = _blocked(v)
    in_maps = []
    for c in range(N_CORES):
        sl = slice(c * HD, (c + 1) * HD)
        in_maps.append({
            "qT": qT,
            "kT": kT,
            "vT": vT,
            "wqT": np.ascontiguousarray(Wq[sl, :].T.astype(np.float16)),
            "wkT": np.ascontiguousarray(Wk[sl, :].T.astype(np.float16)),
            "wvT": np.ascontiguousarray(Wv[sl, :].T.astype(np.float16)),
            "woT": np.ascontiguousarray(Wo[:, c * HD:(c + 1) * HD].T.astype(np.float16)),
            "ones": np.ones((128, 128), dtype=np.float16),
            "bq": np.ascontiguousarray(bq[sl].reshape(HD, 1)),
            "bk": np.ascontiguousarray(bk[sl].reshape(HD, 1)),
        })
    return in_maps


def kernel(q, k, v, Wq, bq, Wk, bk, Wv, bv, Wo, bo):
    global LAST_RESULTS
    from concourse.bass_utils import run_bass_kernel_spmd

    q = np.ascontiguousarray(np.asarray(q, dtype=np.float32))
    k = np.ascontiguousarray(np.asarray(k, dtype=np.float32))
    v = np.ascontiguousarray(np.asarray(v, dtype=np.float32))
    Wq = np.asarray(Wq, dtype=np.float32)
    Wk = np.asarray(Wk, dtype=np.float32)
    Wv = np.asarray(Wv, dtype=np.float32)
    Wo = np.asarray(Wo, dtype=np.float32)
    bq = np.asarray(bq, dtype=np.float32)
    bk = np.asarray(bk, dtype=np.float32)
    bv = np.asarray(bv, dtype=np.float32)
    bo = np.asarray(bo, dtype=np.float32)

    in_maps = _make_in_maps(q, k, v, Wq, bq, Wk, bk, Wv, Wo)

    nc = _get_nc()
    res = run_bass_kernel_spmd(
        nc, in_maps, core_ids=list(range(N_CORES)),
    )
    LAST_RESULTS = res

    attn = np.concatenate([res.results[c]["attn"] for c in range(N_CORES)], axis=0)
    out = np.zeros((BS, D_OUT), dtype=np.float64)
    for c in range(N_CORES):
        out += res.results[c]["outp"]
    # bv folds through softmax (rows sum to 1) into a constant: Wo @ bv + bo
    out += (Wo.astype(np.float64) @ bv.astype(np.float64)) + bo.astype(np.float64)
    return out.astype(np.float32), attn


# revision 30
# speedup vs baseline: 1.0753x; 1.0052x over previous
"""Trainium2 Bass/Tile kernel for multi-head attention (BS=2048, D=1024, H=16, d_k=64).

Returns (output [2048,1024] f32, attn [16,2048,2048] f32) like the reference.
Measured: ~220 us HW exec per core (8 cores), rel err ~5.5e-4 vs fp32 reference.

Sharding: tensor-parallel over heads -- each of the 8 cores owns 2 heads.
Each core reads the full (host-preblocked, fp16-cast) q/k/v plus its head
slices of the weights, computes its 2 heads' attention + attn output, and
writes its slice of `attn` plus a partial output projection.  The host sums
the 8 partials and adds the bias constants (bo + Wo@bv -- bv folds through
the softmax exactly because attention rows sum to 1).

Per-core dataflow (matmul operands fp16, all accumulation fp32):
  - qhT/khT projections land in per-head K-PADDED tiles (the other head's 64
    partitions are zeroed) so every score matmul contracts over a full K=128:
    zeros contribute nothing, and full-array activity keeps the PE clock
    unthrottled (half-array matmuls leave the HAM at K=4/8 = 1.2 GHz).
  - vh (attn@V stationary operand) is built in natural [k-row, head-dim]
    layout with both heads packed in 128 columns.
  - 32 interleaved units (4 q-quarters x 8), each:
      * phase N half-strip per head: S = qhT_z^T @ khT_z -> PSUM [128,1024];
        ScalarE exp(S/8) with fused accum_out row-sums; VectorE reciprocal +
        per-partition tensor_scalar normalize; one merged 1 MB DMA per strip
        writes the finished attention rows.
      * two phase-T mini-strips: S^T = khT_z^T @ qhT_z [128,512] per head;
        exp -> fp16; attn@V accumulates O' [128(both heads),512] per quarter.
  - O' normalization: phase N's per-strip 1/s values are assembled into a
    [1,512] row per (quarter, head) via tiny casting DMAs, broadcast across
    partitions with a K=1 ones matmul, and multiplied in on O' eviction.
  - output projection: full K=128 matmuls (o_sb packs both heads), partial
    fp32 result DMA'd out; host reduces across cores.
  - PSUM budget (8 banks): pn 2x[128,1024] + pt 2x[128,512] + po 2x[128,512].
  - Emission order matters (engines execute their queues in FIFO order):
    k/q-block-0 projections first, vT slabs prefetched, per-quarter q-block
    projections and vh blocks placed where their data is first needed.
"""

import os
import sys
from contextlib import ExitStack

if "/opt/trn_rl_repo" not in sys.path:
    sys.path.insert(0, "/opt/trn_rl_repo")

import numpy as np

BS = 2048
D_IN = 1024
D_OUT = 1024
H = 16
D_K = 64
N_CORES = 8
H_LOC = H // N_CORES          # 2 heads per core
HD = H_LOC * D_K              # 128 head dims per core
KCH = D_IN // 128             # 8 contraction chunks for projections
NSTRIP = BS // 128            # 16 strips of 128
SCALE = 1.0 / np.sqrt(D_K)    # 0.125

_CACHE = {}

# Filled by the last run (for test.py): bass_utils.BassKernelResults
LAST_RESULTS = None


def _build_bass():
    import concourse.bass as bass
    import concourse.tile as tile
    import concourse.mybir as mybir
    from concourse import bacc

    f32 = mybir.dt.float32
    f16 = mybir.dt.float16
    AF = mybir.ActivationFunctionType

    nc = bacc.Bacc(None, target_bir_lowering=False)

    qT = nc.dram_tensor("qT", [4, 128, KCH, 512], f16, kind="ExternalInput")
    kT = nc.dram_tensor("kT", [4, 128, KCH, 512], f16, kind="ExternalInput")
    vT = nc.dram_tensor("vT", [4, 128, KCH, 512], f16, kind="ExternalInput")
    wqT = nc.dram_tensor("wqT", [D_IN, HD], f16, kind="ExternalInput")
    wkT = nc.dram_tensor("wkT", [D_IN, HD], f16, kind="ExternalInput")
    wvT = nc.dram_tensor("wvT", [D_IN, HD], f16, kind="ExternalInput")
    woT = nc.dram_tensor("woT", [HD, D_OUT], f16, kind="ExternalInput")
    onesd = nc.dram_tensor("ones", [128, 128], f16, kind="ExternalInput")
    bq = nc.dram_tensor("bq", [HD, 1], f32, kind="ExternalInput")
    bk = nc.dram_tensor("bk", [HD, 1], f32, kind="ExternalInput")

    attn_out = nc.dram_tensor("attn", [H_LOC, BS, BS], f32, kind="ExternalOutput")
    outp = nc.dram_tensor("outp", [BS, D_OUT], f32, kind="ExternalOutput")

    QB = 512                     # q-quarter width for the T stream
    NQB = BS // QB               # 4 quarters

    with tile.TileContext(nc) as tc, ExitStack() as ctx:
        consts = ctx.enter_context(tc.tile_pool(name="consts", bufs=1))
        slabs = ctx.enter_context(tc.tile_pool(name="slabs", bufs=3))
        vspool = ctx.enter_context(tc.tile_pool(name="vslabs", bufs=4))
        upool = ctx.enter_context(tc.tile_pool(name="u", bufs=6))
        utpool = ctx.enter_context(tc.tile_pool(name="ut", bufs=10))
        apool = ctx.enter_context(tc.tile_pool(name="a", bufs=4))
        spool = ctx.enter_context(tc.tile_pool(name="s", bufs=8))
        outpool = ctx.enter_context(tc.tile_pool(name="outsb", bufs=3))
        orawpool = ctx.enter_context(tc.tile_pool(name="oraw", bufs=3))
        rrowpool = ctx.enter_context(tc.tile_pool(name="rrow", bufs=6))

        # 8-bank PSUM budget:
        #   pn: 2 x [128,1024]f32 (4 banks) -- N scores (one slot per head),
        #       also vh-psum [128,8,128] and outproj [128,1024]
        #   pt: 2 x [128,512]f32 (2 banks)  -- T scores / proj accum / B bcast
        #   po: 2 x [128,512]f32 (2 banks)  -- attn@V accum, both heads packed
        pp_n = ctx.enter_context(tc.tile_pool(name="pp_n", bufs=2, space="PSUM"))
        pp_t = ctx.enter_context(tc.tile_pool(name="pp_t", bufs=2, space="PSUM"))
        pp_o = ctx.enter_context(tc.tile_pool(name="pp_o", bufs=2, space="PSUM"))

        # ---- constants -------------------------------------------------
        w_k = consts.tile([128, KCH, HD], f16, name="w_k", tag="w_k")
        w_q = consts.tile([128, KCH, HD], f16, name="w_q", tag="w_q")
        nc.sync.dma_start(out=w_k, in_=wkT.rearrange("(ko p) m -> p ko m", p=128))
        nc.sync.dma_start(out=w_q, in_=wqT.rearrange("(ko p) m -> p ko m", p=128))
        bq_sb = consts.tile([HD, 1], f32, name="bq_sb", tag="bq_sb")
        bk_sb = consts.tile([HD, 1], f32, name="bk_sb", tag="bk_sb")
        nc.sync.dma_start(out=bk_sb, in_=bk[:, :])
        nc.sync.dma_start(out=bq_sb, in_=bq[:, :])
        w_v = consts.tile([128, KCH, HD], f16, name="w_v", tag="w_v")
        wo_sb = consts.tile([HD, D_OUT], f16, name="wo_sb", tag="wo_sb")
        ones_sb = consts.tile([128, 128], f16, name="ones_sb", tag="ones_sb")

        # per-head K-padded projections: other head's partitions are zero so
        # every score matmul runs with a full K=128 contraction (full-array
        # activity keeps the PE clock unthrottled; zeros contribute nothing)
        qhT_z = [consts.tile([128, BS], f16, name=f"qhT_z{h}", tag=f"qhT_z{h}")
                 for h in range(H_LOC)]
        khT_z = [consts.tile([128, BS], f16, name=f"khT_z{h}", tag=f"khT_z{h}")
                 for h in range(H_LOC)]
        nc.vector.memset(qhT_z[0][D_K:128, :], 0.0)
        nc.vector.memset(qhT_z[1][0:D_K, :], 0.0)
        nc.vector.memset(khT_z[0][D_K:128, :], 0.0)
        nc.vector.memset(khT_z[1][0:D_K, :], 0.0)
        # natural-layout v heads, both packed: cols 0-63 h0, 64-127 h1
        vhb = consts.tile([128, NSTRIP, HD], f16, name="vhb", tag="vhb")
        o_sb = consts.tile([HD, BS], f16, name="o_sb", tag="o_sb")

        # ---- k/q projections, q-block-major ----------------------------
        def proj_block(x_dram, nb, w_sb, b_sb, dst):
                slab = slabs.tile([128, KCH, QB], f16, name="slab", tag="slab")
                nc.sync.dma_start(out=slab, in_=x_dram[nb])
                ps = pp_t.tile([128, QB], f32, name="pj", tag="pt")
                for kc in range(KCH):
                    nc.tensor.matmul(
                        ps,
                        lhsT=w_sb[:, kc, :],
                        rhs=slab[:, kc, :],
                        start=(kc == 0),
                        stop=(kc == KCH - 1),
                    )
                with nc.allow_low_precision("fp16 projections feed fp16 matmul"):
                    nc.vector.tensor_scalar_add(
                        dst[0][0:D_K, nb * QB:(nb + 1) * QB],
                        ps[0:D_K, :], b_sb[0:D_K, 0:1],
                    )
                    nc.vector.tensor_scalar_add(
                        dst[1][D_K:128, nb * QB:(nb + 1) * QB],
                        ps[D_K:128, :], b_sb[D_K:128, 0:1],
                    )

        proj_block(kT, 0, w_k, bk_sb, khT_z)
        proj_block(kT, 1, w_k, bk_sb, khT_z)
        proj_block(qT, 0, w_q, bq_sb, qhT_z)
        proj_block(kT, 2, w_k, bk_sb, khT_z)
        proj_block(kT, 3, w_k, bk_sb, khT_z)

        # ---- vT slabs DMA'd now; vh blocks computed inside the unit loop
        nc.sync.dma_start(out=w_v, in_=wvT.rearrange("(ko p) m -> p ko m", p=128))
        vslabs = []
        for nb in range(4):
            vs = vspool.tile([128, KCH, QB], f16, name="vslab", tag="vslab")
            nc.sync.dma_start(out=vs, in_=vT[nb])
            vslabs.append(vs)
        nc.sync.dma_start(out=wo_sb, in_=woT[:, :])
        nc.sync.dma_start(out=ones_sb, in_=onesd[:, :])

        def vh_block(nb):
            vs = vslabs[nb]
            psv = pp_t.tile([128, 4, 128], f32, name="psv", tag="pt")
            for kc in range(KCH):
                for bx in range(4):
                    nc.tensor.matmul(
                        psv[:, bx, :],
                        lhsT=vs[:, kc, bx * 128:(bx + 1) * 128],
                        rhs=w_v[:, kc, :],
                        start=(kc == 0 and bx % 4 == 0),
                        stop=(kc == KCH - 1),
                        skip_group_check=True,
                    )
            for bx in range(4):
                bc = nb * 4 + bx
                with nc.allow_low_precision("fp16 v-heads feed fp16 matmul"):
                    nc.vector.tensor_copy(
                        out=vhb[:, bc, :],
                        in_=psv[:, bx, :],
                    )

        # ---- attention: 32 units, both heads interleaved ----------------
        def finalize_o(qb, po, rrow0, rrow1):
            """Copy O' (both heads) out of PSUM, normalize by per-head 1/s
            rows broadcast over each head's partition range."""
            o_raw = orawpool.tile([128, QB], f32, name="o_raw", tag="o_raw")
            nc.vector.tensor_copy(out=o_raw, in_=po)
            pb = pp_t.tile([128, QB], f32, name="pb", tag="pt")
            nc.tensor.matmul(
                pb[0:D_K, :], lhsT=ones_sb[0:1, 0:D_K], rhs=rrow0[0:1, :],
                start=True, stop=True, skip_group_check=True,
            )
            nc.tensor.matmul(
                pb[D_K:HD, :], lhsT=ones_sb[0:1, 0:D_K], rhs=rrow1[0:1, :],
                start=True, stop=True, skip_group_check=True,
            )
            with nc.allow_low_precision("O output feeds fp16 out-projection"):
                nc.vector.tensor_mul(
                    o_sb[:, qb * QB:(qb + 1) * QB], o_raw, pb,
                )


        pending = []
        tick = 0
        po_tiles = {}
        rrows = {}
        hold = {}
        for nb in range(4):
            vh_block(nb)
        for nb in range(1, 4):
            proj_block(qT, nb, w_q, bq_sb, qhT_z)
        for qb in range(NQB):
            for un in range(8):
                while pending and pending[0][0] <= tick:
                    finalize_o(*pending.pop(0)[1])
                tick += 1
                ms, khalf = qb * 4 + un // 2, un % 2
                k0 = khalf * 1024
                if un == 0:
                    for hh in range(H_LOC):
                        rrows[(qb, hh)] = rrowpool.tile(
                            [1, QB], f16, name="rrow", tag="rrow")
                # ---- N half-strips, both heads --------------------------
                pn_t = [pp_n.tile([128, 1024], f32, name="pn", tag="pn")
                        for _ in range(H_LOC)]
                for nb in range(2):
                    for h in range(H_LOC):
                        nc.tensor.matmul(
                            pn_t[h][:, nb * 512:(nb + 1) * 512],
                            lhsT=qhT_z[h][:, ms * 128:(ms + 1) * 128],
                            rhs=khT_z[h][:, k0 + nb * 512:k0 + (nb + 1) * 512],
                            start=True,
                            stop=True,
                        )
                for h in range(H_LOC):
                    u = upool.tile([128, 1024], f32, name="u", tag="u")
                    sp = spool.tile([128, 1], f32, name="sp", tag=f"sp{khalf}{h}")
                    nc.scalar.activation(
                        out=u, in_=pn_t[h], func=AF.Exp, scale=float(SCALE),
                        accum_out=sp,
                    )
                    if khalf == 0:
                        hold[h] = (u, sp)
                    else:
                        u0, s0 = hold[h]
                        s = spool.tile([128, 1], f32, name="s", tag=f"s{h}")
                        nc.vector.tensor_add(s, s0, sp)
                        r = spool.tile([128, 1], f32, name="r", tag=f"r{h}")
                        nc.vector.reciprocal(out=r, in_=s)
                        nc.gpsimd.dma_start(
                            out=rrows[(qb, h)][0:1,
                                               (ms % 4) * 128:(ms % 4 + 1) * 128],
                            in_=r[:, 0:1],
                        )
                        a = apool.tile([128, BS], f32, name="a", tag="a")
                        for uu, kh in ((u0, 0), (u, 1)):
                            nc.vector.tensor_scalar_mul(
                                a[:, kh * 1024:(kh + 1) * 1024], uu, r[:, 0:1])
                        nc.sync.dma_start(
                            out=attn_out[h, ms * 128:(ms + 1) * 128, :], in_=a,
                        )
                # ---- two phase-T mini-strips (q-quarter qb) -------------
                q0 = qb * QB
                for ks in (2 * un, 2 * un + 1):
                    if ks == 0:
                        po_tiles[qb] = pp_o.tile(
                            [128, QB], f32, name="po", tag="po")
                    po = po_tiles[qb]
                    pt_t = [pp_t.tile([128, QB], f32, name="pt", tag="pt")
                            for _ in range(H_LOC)]
                    for hh in range(H_LOC):
                        nc.tensor.matmul(
                            pt_t[hh],
                            lhsT=khT_z[hh][:, ks * 128:(ks + 1) * 128],
                            rhs=qhT_z[hh][:, q0:q0 + QB],
                            start=True,
                            stop=True,
                        )
                    uts = []
                    for hh in range(H_LOC):
                        ut = utpool.tile([128, QB], f16, name="ut", tag="ut")
                        nc.scalar.activation(
                            out=ut, in_=pt_t[hh], func=AF.Exp, scale=float(SCALE))
                        uts.append(ut)
                    for hh in range(H_LOC):
                        nc.tensor.matmul(
                            po[hh * D_K:(hh + 1) * D_K, :],
                            lhsT=vhb[:, ks, hh * D_K:(hh + 1) * D_K],
                            rhs=uts[hh],
                            start=(ks == 0),
                            stop=(ks == NSTRIP - 1),
                            skip_group_check=True,
                        )
                    if ks == NSTRIP - 1:
                        pending.append(
                            (tick + (6 if qb < NQB - 1 else 0),
                             (qb, po, rrows[(qb, 0)], rrows[(qb, 1)])))
        def outproj(bc):
            pout = pp_n.tile([128, 1024], f32, name="pout", tag="pn")
            for oc in range(2):
                nc.tensor.matmul(
                    pout[:, oc * 512:(oc + 1) * 512],
                    lhsT=o_sb[:, bc * 128:(bc + 1) * 128],
                    rhs=wo_sb[:, oc * 512:(oc + 1) * 512],
                    start=True,
                    stop=True,
                )
            osb = outpool.tile([128, 1024], f32, name="osb", tag="osb")
            if bc % 2 == 0:
                nc.vector.tensor_copy(out=osb, in_=pout)
            else:
                nc.scalar.copy(out=osb, in_=pout)
            nc.sync.dma_start(out=outp[bc * 128:(bc + 1) * 128, :], in_=osb)

        # quarters 0-2 are finalized already; emit their projection first so
        # it overlaps the last quarter's finalize chain
        n_ready = (NQB - len(pending)) * 4
        for bc in range(n_ready):
            outproj(bc)
        while pending:
            finalize_o(*pending.pop(0)[1])
        for bc in range(n_ready, NSTRIP):
            outproj(bc)

    nc.compile()
    return nc


def _get_nc():
    if "nc" not in _CACHE:
        _CACHE["nc"] = _build_bass()
    return _CACHE["nc"]


def _make_in_maps(q, k, v, Wq, bq, Wk, bk, Wv, Wo):
    def _blocked(x):
        # [BS, D_IN] -> [4, 128, 8, 512]: slab nb holds x^T chunk
        # [p, ko, n] = x[nb*512+n, ko*128+p]
        return np.ascontiguousarray(
            x.astype(np.float16).reshape(4, 512, KCH, 128).transpose(0, 3, 2, 1))

    qT = _blocked(q)
    kT = _blocked(k)
    vT = _blocked(v)
    in_maps = []
    for c in range(N_CORES):
        sl = slice(c * HD, (c + 1) * HD)
        in_maps.append({
            "qT": qT,
            "kT": kT,
            "vT": vT,
            "wqT": np.ascontiguousarray(Wq[sl, :].T.astype(np.float16)),
            "wkT": np.ascontiguousarray(Wk[sl, :].T.astype(np.float16)),
            "wvT": np.ascontiguousarray(Wv[sl, :].T.astype(np.float16)),
            "woT": np.ascontiguousarray(Wo[:, c * HD:(c + 1) * HD].T.astype(np.float16)),
            "ones": np.ones((128, 128), dtype=np.float16),
            "bq": np.ascontiguousarray(bq[sl].reshape(HD, 1)),
            "bk": np.ascontiguousarray(bk[sl].reshape(HD, 1)),
        })
    return in_maps


def kernel(q, k, v, Wq, bq, Wk, bk, Wv, bv, Wo, bo):
    global LAST_RESULTS
    from concourse.bass_utils import run_bass_kernel_spmd

    q = np.ascontiguousarray(np.asarray(q, dtype=np.float32))
    k = np.ascontiguousarray(np.asarray(k, dtype=np.float32))
    v = np.ascontiguousarray(np.asarray(v, dtype=np.float32))
    Wq = np.asarray(Wq, dtype=np.float32)
    Wk = np.asarray(Wk, dtype=np.float32)
    Wv = np.asarray(Wv, dtype=np.float32)
    Wo = np.asarray(Wo, dtype=np.float32)
    bq = np.asarray(bq, dtype=np.float32)
    bk = np.asarray(bk, dtype=np.float32)
    bv = np.asarray(bv, dtype=np.float32)
    bo = np.asarray(bo, dtype=np.float32)

    in_maps = _make_in_maps(q, k, v, Wq, bq, Wk, bk, Wv, Wo)

    nc = _get_nc()
    res = run_bass_kernel_spmd(
        nc, in_maps, core_ids=list(range(N_CORES)),
    )
    LAST_RESULTS = res

    attn = np.concatenate([res.results[c]["attn"] for c in range(N_CORES)], axis=0)
    out = np.zeros((BS, D_OUT), dtype=np.float64)
    for c in range(N_CORES):
        out += res.results[c]["outp"]
    # bv folds through softmax (rows sum to 1) into a constant: Wo @ bv + bo
    out += (Wo.astype(np.float64) @ bv.astype(np.float64)) + bo.astype(np.float64)
    return out.astype(np.float32), attn
